# revision 18
# baseline (speedup 1.0000x reference)
"""GAT (2-layer, 4->1 heads) + global mean pool + classifier on 8 trn2 NeuronCores.

Sharding: nodes (and their incoming edges) partitioned contiguously across the
8 cores; small weights replicated; per-layer node-feature tables AllGathered
(in 4 chunks, overlapped with the producing phase); per-graph pooled sums
AllReduced.

Edge phase: per-edge feature rows pulled from the AllGathered DRAM table with
single-packet dma_gather calls (<=1024 indices each, rotated over 4 SWDGE
queues) so the SDMA engines pipeline the random HBM reads. The aggregation
one-hot is precomputed on the host and bulk-DMA'd; softmax weights ride along
the gathered rows and scale messages with a 2x-packed DVE multiply.

Self-contained: takes full inputs, returns full [64, 2] log-softmax output.
"""
import sys
for _p in ('/opt/trn_rl_repo', '/root/.axon_site/_ro/trn_rl_repo'):
    if _p not in sys.path:
        sys.path.insert(0, _p)

import heapq
import numpy as np
import concourse.bass as bass
import concourse.bacc as bacc
import concourse.tile as tile
import concourse.mybir as mybir
from concourse import bass_utils, library_config

dt = mybir.dt

# problem constants (hardcoded per contract); N/E re-derivable for sim tests
N = 50000
E = 1600000
G = 64
DIN = 128
HID = 64
H = 4
NEG_SLOPE = 0.2
BN_EPS = 1e-5
NC = 8
RF1 = 512                # L1 table row: fp8 slots (512B): 256 xws fp8 | 16B a_src(f32) | pad
RF2 = 128                # L2 table row: fp16 slots (256B): 64 xws | 2 a_src(f32) | pad
SPCH = 8                 # chunks per dma_gather call (8*128=1024 idx: 64-desc single packet max)
NAG = 4                  # AllGather chunks


def configure(n, e):
    global N, E, NSH, NT, NPAD, HALF, GRP, ROWS_G, BASE_G
    N, E = n, e
    NSH = N // NC
    NT = (NSH + 127) // 128
    NPAD = NT * 128
    HALF = NC // 2 * NPAD
    # AllGather tile groups, front-loaded so the final chunk (which gates the
    # next phase) is small
    base = -(-NT * 10 // (NAG * 10))
    big = min(NT - (NAG - 1), base + (base + 1) // 4)
    sizes = [big] * (NAG - 1) + [NT - big * (NAG - 1)]
    assert sizes[-1] >= 1
    GRP = [0]
    for s in sizes:
        GRP.append(GRP[-1] + s)
    ROWS_G = [(GRP[i + 1] - GRP[i]) * 128 for i in range(NAG)]
    BASE_G = [0]
    for rg in ROWS_G:
        BASE_G.append(BASE_G[-1] + NC * rg)


configure(N, E)

_cache = {}


def _balanced_assign(indeg):
    """Greedy LPT: nodes -> (core,tile) buckets, balancing edge counts.

    Returns newid[node] = core*NSH + tile*128 + slot."""
    nbuck = NC * NT
    lastcap = NSH - (NT - 1) * 128
    caps = np.full(nbuck, 128, np.int64)
    caps[NT - 1::NT] = lastcap
    order = np.argsort(-indeg, kind='stable')
    load = [0] * nbuck
    fill = [0] * nbuck
    heap = [(0, b) for b in range(nbuck)]
    heapq.heapify(heap)
    newid = np.empty(N, np.int64)
    caps_l = caps.tolist()
    indeg_l = indeg.tolist()
    for node in order.tolist():
        while True:
            l, b = heapq.heappop(heap)
            if fill[b] < caps_l[b]:
                break
        s = fill[b]
        fill[b] = s + 1
        nl = l + indeg_l[node]
        load[b] = nl
        if s + 1 < caps_l[b]:
            heapq.heappush(heap, (nl, b))
        c, t = divmod(b, NT)
        newid[node] = c * NSH + t * 128 + s
    return newid


def _prep_host(x, edge_index, batch,
               W1, att_src1, att_dst1, bias1, bn1_g, bn1_b, bn1_m, bn1_v,
               W2, att_src2, att_dst2, bias2, bn2_g, bn2_b, bn2_m, bn2_v,
               Wc1, bc1, Wc2, bc2):
    """Index-space layout + folded weights. Returns (in_maps, CH, choff, CHUNKS)."""
    f32 = np.float32
    src = np.concatenate([np.asarray(edge_index[0], np.int64),
                          np.arange(N, dtype=np.int64)])
    dst = np.concatenate([np.asarray(edge_index[1], np.int64),
                          np.arange(N, dtype=np.int64)])
    EE = src.shape[0]

    indeg = np.bincount(dst, minlength=N)
    newid = _balanced_assign(indeg)
    inv = np.empty(N, np.int64)
    inv[newid] = np.arange(N)      # inv[new] = old
    src = newid[src]
    dst = newid[dst]

    core = dst // NSH
    ldst = dst - core * NSH
    t = ldst >> 7                     # dst tile within shard
    dit = ldst & 127                  # dst index within tile

    # table row of src in the group-major AllGathered table
    score = src // NSH
    sr = src - score * NSH            # local row
    stile = sr >> 7
    g_of_tile = np.zeros(NT, np.int64)
    for gi in range(NAG):
        g_of_tile[GRP[gi]:GRP[gi + 1]] = gi
    sg = g_of_tile[stile]
    rows_g = np.asarray(ROWS_G, np.int64)
    base_g = np.asarray(BASE_G[:NAG], np.int64)
    grp0 = np.asarray([GRP[i] * 128 for i in range(NAG)], np.int64)
    trow = base_g[sg] + score * rows_g[sg] + (sr - grp0[sg])
    g = (trow >= HALF).astype(np.int64)           # table half (int16 range)
    lidx = trow - g * HALF
    assert lidx.max() < 32768

    key = ((core * NT + t) * 2 + g)   # bucket id, core-major
    nbuck = NC * NT * 2
    cnt = np.bincount(key, minlength=nbuck).reshape(NC, NT, 2)
    CH = np.maximum(1, (cnt.max(axis=0) + 127) // 128)   # [NT, 2] shared chunks
    CHUNKS = int(CH.sum())
    choff = np.zeros((NT, 2), np.int64)
    choff.reshape(-1)[1:] = np.cumsum(CH.reshape(-1))[:-1]
    CH4 = CH
    choff4 = choff

    # stable-sort edges by bucket; ranks within bucket
    order = np.argsort(key, kind='stable')
    skey = key[order]
    bstart = np.searchsorted(skey, np.arange(nbuck))
    rank = np.arange(EE, dtype=np.int64) - bstart[skey]
    bt = (skey // 2) % NT
    bg = skey % 2
    pos = choff[bt, bg] * 128 + rank
    scor = skey // (NT * 2)

    gidx_all = np.zeros((NC, CHUNKS * 128), np.int16)
    dflat_all = np.full((NC, CHUNKS * 128), 999.0, np.float16)
    didx_all = np.full((NC, CHUNKS * 128), 999, np.int32)
    for c in range(NC):
        m = scor == c
        eidx = order[m]
        gidx_all[c, pos[m]] = lidx[eidx].astype(np.int16)
        dflat_all[c, pos[m]] = dit[eidx].astype(np.float16)
        didx_all[c, pos[m]] = dit[eidx]

    # gather index stream: wrapped in 16 partitions, replicated x8
    gidx = np.tile(gidx_all.reshape(NC, CHUNKS * 8, 16).transpose(0, 2, 1), (1, 8, 1)).copy()
    # host-built one-hots (fp8: 0.0/1.0 are exact)
    #   oh[p, ch, j]  = (dit of edge (p,ch) == j)     aggregation lhsT
    #   ohT[d, ch, e] = (dit of edge (e,ch) == d)     alpha_dst lhsT
    import ml_dtypes
    f8 = ml_dtypes.float8_e4m3fn
    ar128 = np.arange(128, dtype=np.int32)
    oh = np.empty((NC, 128, CHUNKS, 128), f8)
    ohT = np.empty((NC, 128, CHUNKS, 128), f8)
    for c in range(NC):
        dit_pc = didx_all[c].reshape(CHUNKS, 128)            # [ch, p]
        oh[c] = (dit_pc.T[:, :, None] == ar128[None, None, :]).astype(f8)
        ohT[c] = (ar128[:, None, None] == dit_pc[None, :, :]).astype(f8)

    # batch / pooling (note: node n' holds old node inv[n'])
    batch = np.asarray(batch, np.int64)[inv]
    bcol = np.full((NC, 128, NT), 999.0, np.float16)
    for c in range(NC):
        bc_ = batch[c * NSH:(c + 1) * NSH].astype(np.float16)
        pad = np.full(NPAD - NSH, 999.0, np.float16)
        bcol[c] = np.concatenate([bc_, pad]).reshape(NT, 128).T
    cnt_g = np.bincount(batch, minlength=G).astype(f32)
    cntrecip = (1.0 / np.maximum(cnt_g, 1.0)).reshape(G, 1)

    # folded weights
    W1 = np.asarray(W1, f32); W2 = np.asarray(W2, f32)
    s1 = np.asarray(bn1_g, f32) / np.sqrt(np.asarray(bn1_v, f32) + BN_EPS)
    t1 = (np.asarray(bias1, f32) - np.asarray(bn1_m, f32)) * s1 + np.asarray(bn1_b, f32)
    s2 = np.asarray(bn2_g, f32) / np.sqrt(np.asarray(bn2_v, f32) + BN_EPS)
    t2 = (np.asarray(bias2, f32) - np.asarray(bn2_m, f32)) * s2 + np.asarray(bn2_b, f32)
    aS1 = np.asarray(att_src1, f32)   # [H, HID]
    aD1 = np.asarray(att_dst1, f32)
    Ablk = np.zeros((H * HID, 2 * H), f32)
    for h in range(H):
        Ablk[h * HID:(h + 1) * HID, h] = aS1[h]
        Ablk[h * HID:(h + 1) * HID, H + h] = aD1[h]
    W1e = np.concatenate([W1 * s1[None, :], W1 @ Ablk], axis=1)      # [128, 264]
    t1row = np.concatenate([t1, np.zeros(2 * H, f32)]).reshape(1, 264)
    aS2 = np.asarray(att_src2, f32).reshape(HID)
    aD2 = np.asarray(att_dst2, f32).reshape(HID)
    W2e = np.concatenate([W2 * s2[None, :], (W2 @ aS2)[:, None],
                          (W2 @ aD2)[:, None]], axis=1)              # [256, 66]
    t2row = np.concatenate([t2, np.zeros(2, f32)]).reshape(1, 66)

    iota64 = np.tile(np.arange(64, dtype=np.float16), (128, 1))
    ident = np.eye(128, dtype=f32)
    onesrow = np.ones((1, 128), f32)

    x = np.asarray(x, f32)[inv]
    in_maps = []
    for c in range(NC):
        xs = x[c * NSH:(c + 1) * NSH]
        xT = np.zeros((DIN, NPAD), f32)
        xT[:, :NSH] = xs.T
        in_maps.append({
            "xT": xT, "gidx": gidx[c],
            "oh": oh[c], "ohT": ohT[c],
            "bcol": bcol[c].copy(),
            "W1e": W1e, "t1row": t1row,
            "W2e": W2e.reshape(2, 128, 66).transpose(1, 0, 2).copy(),
            "t2row": t2row,
            "iota64": iota64, "ident": ident, "onesrow": onesrow,
            "Wc1": np.asarray(Wc1, f32), "bc1row": np.asarray(bc1, f32).reshape(1, HID),
            "Wc2": np.asarray(Wc2, f32), "bc2row": np.asarray(bc2, f32).reshape(1, 2),
            "cntrecip": cntrecip,
        })
    return in_maps, CH, choff, CH4, choff4, CHUNKS


def _build(CH, choff, CH4, choff4, CHUNKS):
    AluOp = mybir.AluOpType
    Act = mybir.ActivationFunctionType
    nc = bacc.Bacc("TRN2", target_bir_lowering=False, debug=False, num_devices=NC,
                   num_swdge_queues=4)

    xT_d = nc.dram_tensor("xT", [DIN, NPAD], dt.float32, kind="ExternalInput")
    gidx_d = nc.dram_tensor("gidx", [128, CHUNKS * 8], dt.int16, kind="ExternalInput")
    oh_d = nc.dram_tensor("oh", [128, CHUNKS, 128], dt.float8e4, kind="ExternalInput")
    ohT_d = nc.dram_tensor("ohT", [128, CHUNKS, 128], dt.float8e4, kind="ExternalInput")
    bcol_d = nc.dram_tensor("bcol", [128, NT], dt.float16, kind="ExternalInput")
    W1e_d = nc.dram_tensor("W1e", [DIN, 264], dt.float32, kind="ExternalInput")
    t1row_d = nc.dram_tensor("t1row", [1, 264], dt.float32, kind="ExternalInput")
    W2e_d = nc.dram_tensor("W2e", [128, 2, 66], dt.float32, kind="ExternalInput")
    t2row_d = nc.dram_tensor("t2row", [1, 66], dt.float32, kind="ExternalInput")
    iota64_d = nc.dram_tensor("iota64", [128, 64], dt.float16, kind="ExternalInput")
    ident_d = nc.dram_tensor("ident", [128, 128], dt.float32, kind="ExternalInput")
    ones_d = nc.dram_tensor("onesrow", [1, 128], dt.float32, kind="ExternalInput")
    Wc1_d = nc.dram_tensor("Wc1", [HID, HID], dt.float32, kind="ExternalInput")
    bc1_d = nc.dram_tensor("bc1row", [1, HID], dt.float32, kind="ExternalInput")
    Wc2_d = nc.dram_tensor("Wc2", [HID, 2], dt.float32, kind="ExternalInput")
    bc2_d = nc.dram_tensor("bc2row", [1, 2], dt.float32, kind="ExternalInput")
    crec_d = nc.dram_tensor("cntrecip", [G, 1], dt.float32, kind="ExternalInput")
    out_d = nc.dram_tensor("out", [G, 2], dt.float32, kind="ExternalOutput")

    RG = [list(range(NC))]
    CTMAX = int((CH[:, 0] + CH[:, 1]).max())
    qrr = [0]

    with tile.TileContext(nc) as tc:
        with (
            tc.tile_pool(name="const", bufs=1) as cp,
            tc.tile_pool(name="sb", bufs=3) as sb,
            tc.tile_pool(name="gbuf", bufs=4) as gp,
            tc.tile_pool(name="ohbuf", bufs=4) as ohp,
            tc.tile_pool(name="gbuf2", bufs=2) as gp2,
            tc.tile_pool(name="small", bufs=4) as sp,
            tc.tile_pool(name="ps", bufs=2, space="PSUM") as ps,
            tc.tile_pool(name="pspool", bufs=1, space="PSUM") as psp,
            tc.tile_pool(name="dram", bufs=1, space="DRAM") as dram,
        ):
            nc.gpsimd.load_library(library_config.mlp)

            # ---- consts to SBUF
            def cload(dten, shape, dtype):
                tl = cp.tile(shape, dtype, tag=dten.name)
                nc.sync.dma_start(tl[:], dten[:])
                return tl
            W1e = cload(W1e_d, [DIN, 264], dt.float32)
            t1row = cload(t1row_d, [1, 264], dt.float32)
            W2e = cload(W2e_d, [128, 2, 66], dt.float32)
            t2row = cload(t2row_d, [1, 66], dt.float32)
            iota64 = cload(iota64_d, [128, 64], dt.float16)
            ident = cload(ident_d, [128, 128], dt.float32)
            ones = cload(ones_d, [1, 128], dt.float32)
            Wc1 = cload(Wc1_d, [HID, HID], dt.float32)
            bc1row = cload(bc1_d, [1, HID], dt.float32)
            Wc2 = cload(Wc2_d, [HID, 2], dt.float32)
            bc2row = cload(bc2_d, [1, 2], dt.float32)
            cntrecip = cload(crec_d, [G, 1], dt.float32)
            bcol = cload(bcol_d, [128, NT], dt.float16)

            ad1 = cp.tile([128, NT, H], dt.float16, tag="ad1")
            ad2 = cp.tile([128, NT, 1], dt.float16, tag="ad2")
            poh = cp.tile([128, NT, G], dt.float16, tag="poh")
            # pooling one-hot (built once)
            nc.vector.tensor_tensor(
                poh[:],
                iota64[:].unsqueeze(1).broadcast_to([128, NT, G]),
                bcol[:].unsqueeze(2).broadcast_to([128, NT, G]),
                AluOp.is_equal)

            # ---- DRAM tables (collective outputs in Shared space)
            t1stage = dram.tile([NPAD, RF1], dt.float8e4)
            t1full = nc.dram_tensor("t1full", [NC * NPAD, RF1], dt.float8e4,
                                    kind="Internal", addr_space="Shared").ap()
            t2stage = dram.tile([NPAD, RF2], dt.float16)
            t2full = nc.dram_tensor("t2full", [NC * NPAD, RF2], dt.float16,
                                    kind="Internal", addr_space="Shared").ap()

            def ag_chunk(stage, full, rfw, gi):
                a, b = GRP[gi] * 128, GRP[gi + 1] * 128
                base = BASE_G[gi]
                nc.gpsimd.collective_compute(
                    "AllGather", mybir.AluOpType.bypass, replica_groups=RG,
                    ins=[stage[a:b, :].opt()],
                    outs=[full[base:base + NC * (b - a), :].opt()])

            # ================= PHASE A: L1 head (xw1 table + alphas) ======
            with nc.named_scope("phaseA"), tc.tile_pool(name="head", bufs=2) as hp:
                for gi in range(NAG):
                    a, b = GRP[gi], GRP[gi + 1]
                    xTg = hp.tile([DIN, (b - a) * 128], dt.float32, tag="xTg")
                    nc.sync.dma_start(xTg[:], xT_d[:, a * 128:b * 128])
                    for t in range(a, b):
                        pa = ps.tile([128, 264], dt.float32, tag="pb")
                        nc.tensor.matmul(pa[:], xTg[:, (t - a) * 128:(t - a + 1) * 128],
                                         W1e[:],
                                         start=True, stop=False)
                        nc.tensor.matmul(pa[:], ones[0:1, :], t1row[:],
                                         start=False, stop=True)
                        tab = sb.tile([128, RF1], dt.float8e4, tag="tab1")
                        nc.scalar.activation(tab[:, 0:256], pa[:, 0:256], Act.Copy)
                        nc.vector.tensor_copy(tab[:, 256:272].bitcast(dt.float32),
                                              pa[:, 256:260])
                        nc.vector.tensor_copy(ad1[:, t, :], pa[:, 260:264])
                        nc.vector.memset(tab[:, 272:RF1].bitcast(dt.float16), 0.0)
                        nc.sync.dma_start(t1stage[t * 128:(t + 1) * 128, :], tab[:])
                    ag_chunk(t1stage, t1full, RF1, gi)

            # ================= PHASE B: L1 edges + L2 head ================
            def edge_phase(layer, tfull, rfw, nh, adt, adrow_tag):
                """One GAT edge phase. Yields per-tile (t, h) results."""
                halves = (tfull[0:HALF, :], tfull[HALF:2 * HALF, :])
                ncol = nh * HID   # message feature cols (256 / 64)
                fp8 = layer == 1
                for t in range(NT):
                    ct0 = int(choff[t, 0]); n0 = int(CH[t, 0])
                    n1 = int(CH[t, 1])
                    ctot = n0 + n1

                    # stage this tile's gather indices + host one-hots
                    gix = sp.tile([128, CTMAX * 8], dt.int16, tag="gix")
                    nc.sync.dma_start(gix[:, 0:ctot * 8],
                                      gidx_d[:, ct0 * 8:(ct0 + ctot) * 8])
                    oht = ohp.tile([128, CTMAX, 128], dt.float8e4, tag="oht")
                    nc.sync.dma_start(oht[:, 0:ctot, :],
                                      oh_d[:, ct0:ct0 + ctot, :])
                    ohTt = ohp.tile([128, CTMAX, 128], dt.float8e4, tag="ohTt")
                    nc.sync.dma_start(ohTt[:, 0:ctot, :],
                                      ohT_d[:, ct0:ct0 + ctot, :])

                    gb = gp.tile([128, CTMAX, rfw],
                                 dt.float8e4 if fp8 else dt.float16, tag="gb")
                    for gi, nch in ((0, n0), (1, n1)):
                        boff = 0 if gi == 0 else n0
                        for so in range(0, nch, SPCH):
                            ns = min(SPCH, nch - so)
                            nc.gpsimd.dma_gather(
                                gb[:, boff + so:boff + so + ns, :], halves[gi],
                                gix[:, (boff + so) * 8:(boff + so + ns) * 8],
                                num_idxs=ns * 128, num_idxs_reg=ns * 128,
                                elem_size=rfw, queue_num=qrr[0] % 4,
                                single_packet=True)
                            qrr[0] += 1
                    if fp8:
                        # cast fp8 features -> f16 message buffer on ACT
                        mb = gp2.tile([128, CTMAX, ncol + nh], dt.float16,
                                      tag="mbuf")
                        nc.scalar.activation(mb[:, 0:ctot, 0:ncol],
                                             gb[:, 0:ctot, 0:ncol], Act.Copy)
                        asrc = gb[:, 0:ctot, ncol:ncol + 4 * nh].bitcast(dt.float32)
                    else:
                        mb = gb
                        asrc = gb[:, 0:ctot, ncol:ncol + 2 * nh].bitcast(dt.float32)
                    ade = ps.tile([128, CTMAX, nh], dt.float32, tag="pexp")
                    for c in range(ctot):
                        nc.tensor.matmul(
                            ade[:, c, :],
                            ohTt[:, c, :], adt[:, t, :],
                            start=True, stop=True)
                    # e = a_src + a_dst ; lrelu ; exp -> fp16 into mb
                    ee = sp.tile([128, CTMAX, nh], dt.float32, tag=f"ee{layer}")
                    nc.vector.tensor_tensor(
                        ee[:, 0:ctot, :], asrc,
                        ade[:, 0:ctot, :], AluOp.add)
                    nc.vector.scalar_tensor_tensor(
                        ee[:, 0:ctot, :], ee[:, 0:ctot, :], NEG_SLOPE,
                        ee[:, 0:ctot, :], AluOp.mult, AluOp.max)
                    nc.scalar.activation(mb[:, 0:ctot, ncol:ncol + nh],
                                         ee[:, 0:ctot, :], Act.Exp)
                    # duplicate weights for a 2x-packed multiply
                    expd = sp.tile([128, CTMAX, nh, 2], dt.float16, tag=f"xd{layer}")
                    nc.vector.tensor_copy(
                        expd[:, 0:ctot, :, :],
                        mb[:, 0:ctot, ncol:ncol + nh].unsqueeze(3)
                          .broadcast_to([128, ctot, nh, 2]))
                    # scale messages by exp (in place, pair-packed APs, per head)
                    for h in range(nh):
                        nc.vector.tensor_tensor(
                            mb[:, 0:ctot, h * HID:(h + 1) * HID].rearrange(
                                "p c (j two) -> p c j two", two=2),
                            mb[:, 0:ctot, h * HID:(h + 1) * HID].rearrange(
                                "p c (j two) -> p c j two", two=2),
                            expd[:, 0:ctot, h, :].unsqueeze(2)
                              .broadcast_to([128, ctot, HID // 2, 2]),
                            AluOp.mult)
                    # aggregate
                    pb = ps.tile([128, ncol + nh], dt.float32, tag="pagg")
                    for c in range(ctot):
                        nc.tensor.matmul(pb[:], oht[:, c, :],
                                         mb[:, c, 0:ncol + nh],
                                         start=(c == 0), stop=(c == ctot - 1))
                    # epilogue: h = relu(num / (den + eps))
                    den = sp.tile([128, nh], dt.float32, tag=f"den{layer}")
                    nc.vector.tensor_scalar(den[:], pb[:, ncol:ncol + nh],
                                            1e-16, None, AluOp.add)
                    rec = sp.tile([128, nh], dt.float32, tag=f"rec{layer}")
                    nc.vector.reciprocal(rec[:], den[:])
                    hsb = sb.tile([128, ncol], dt.float32, tag=f"h{layer}")
                    nc.vector.tensor_tensor(
                        hsb[:].rearrange("p (h f) -> p h f", h=nh),
                        pb[:, 0:ncol].rearrange("p (h f) -> p h f", h=nh),
                        rec[:].unsqueeze(2).broadcast_to([128, nh, HID]),
                        AluOp.mult)
                    nc.scalar.activation(hsb[:], hsb[:], Act.Relu)
                    yield t, hsb

            # L1 edge phase; fused L2 head per tile, AG2 chunked
            ag2_next = 0
            for t, h1 in edge_phase(1, t1full, RF1, H, ad1, "a1"):
                # transpose h1 [128, 256] -> two [128,128] slices
                h1T = sb.tile([128, 2, 128], dt.float32, tag="h1T")
                for k in range(2):
                    pt = ps.tile([128, 128], dt.float32, tag="pb")
                    nc.tensor.transpose(pt[:], h1[:, k * 128:(k + 1) * 128],
                                        ident[:])
                    nc.scalar.copy(h1T[:, k, :], pt[:])
                pc = ps.tile([128, 66], dt.float32, tag="pb")
                for k in range(2):
                    nc.tensor.matmul(pc[:], h1T[:, k, :], W2e[:, k, :],
                                     start=(k == 0), stop=False)
                nc.tensor.matmul(pc[:], ones[0:1, :], t2row[:],
                                 start=False, stop=True)
                tab2 = sb.tile([128, RF2], dt.float16, tag="tab2")
                nc.scalar.activation(tab2[:, 0:HID], pc[:, 0:HID], Act.Copy)
                nc.vector.tensor_copy(tab2[:, HID:HID + 2].bitcast(dt.float32),
                                      pc[:, HID:HID + 1])
                nc.vector.tensor_copy(ad2[:, t, :], pc[:, HID + 1:HID + 2])
                nc.vector.memset(tab2[:, HID + 2:RF2], 0.0)
                nc.sync.dma_start(t2stage[t * 128:(t + 1) * 128, :], tab2[:])
                while (ag2_next < NAG - 1
                       and t >= min(GRP[ag2_next + 1] - 1 + 4, NT - 1)):
                    ag_chunk(t2stage, t2full, RF2, ag2_next)
                    ag2_next += 1
            while ag2_next < NAG:
                ag_chunk(t2stage, t2full, RF2, ag2_next)
                ag2_next += 1

            # ================= PHASE C: L2 edges + pooling ================
            pgsum = psp.tile([G, HID], dt.float32, tag="pgsum")
            for t, h2 in edge_phase(2, t2full, RF2, 1, ad2, "a2"):
                h2h = sb.tile([128, HID], dt.float16, tag="h2h")
                nc.scalar.activation(h2h[:], h2[:], Act.Copy)
                nc.tensor.matmul(pgsum[:], poh[:, t, :], h2h[:],
                                 start=(t == 0), stop=(t == NT - 1))

            # ================= PHASE D: AllReduce + classifier ============
            ar_in = dram.tile([G, HID], dt.float32)
            ar_out = nc.dram_tensor("ar_out", [G, HID], dt.float32,
                                    kind="Internal", addr_space="Shared").ap()
            psum_sb = sb.tile([G, HID], dt.float32, tag="psum_sb")
            nc.vector.tensor_copy(psum_sb[:], pgsum[:])
            nc.sync.dma_start(ar_in[:], psum_sb[:])
            nc.gpsimd.collective_compute(
                "AllReduce", mybir.AluOpType.add, replica_groups=RG,
                ins=[ar_in.opt()], outs=[ar_out.opt()])
            rep = sb.tile([G, HID], dt.float32, tag="rep")
            nc.sync.dma_start(rep[:], ar_out[:])
            nc.vector.tensor_scalar(rep[:], rep[:], cntrecip[:, 0:1], None,
                                    AluOp.mult)
            # hc = relu(rep @ Wc1 + bc1)
            ptr = ps.tile([G, G], dt.float32, tag="pb")
            nc.tensor.transpose(ptr[:], rep[:], ident[0:G, 0:G])
            repT = sb.tile([G, G], dt.float32, tag="repT")
            nc.scalar.copy(repT[:], ptr[:])
            ph = ps.tile([G, HID], dt.float32, tag="pb")
            nc.tensor.matmul(ph[:], repT[:], Wc1[:], start=True, stop=False)
            nc.tensor.matmul(ph[:], ones[0:1, 0:G], bc1row[:],
                             start=False, stop=True)
            hc = sb.tile([G, HID], dt.float32, tag="hc")
            nc.scalar.activation(hc[:], ph[:], Act.Relu)
            pt2 = ps.tile([G, G], dt.float32, tag="pb")
            nc.tensor.transpose(pt2[:], hc[:], ident[0:G, 0:G])
            hcT = sb.tile([G, G], dt.float32, tag="hcT")
            nc.scalar.copy(hcT[:], pt2[:])
            pl = ps.tile([G, 2], dt.float32, tag="pb")
            nc.tensor.matmul(pl[:], hcT[:], Wc2[:], start=True, stop=False)
            nc.tensor.matmul(pl[:], ones[0:1, 0:G], bc2row[:],
                             start=False, stop=True)
            # log softmax over the 2 logits
            lg = sb.tile([G, 2], dt.float32, tag="lg")
            nc.vector.tensor_copy(lg[:], pl[:])
            mx = sb.tile([G, 1], dt.float32, tag="mx")
            nc.vector.tensor_reduce(mx[:], lg[:], mybir.AxisListType.X,
                                    AluOp.max)
            nc.vector.tensor_scalar(lg[:], lg[:], mx[:, 0:1], None,
                                    AluOp.subtract)
            ex = sb.tile([G, 2], dt.float32, tag="ex")
            nc.scalar.activation(ex[:], lg[:], Act.Exp)
            sm = sb.tile([G, 1], dt.float32, tag="sm")
            nc.vector.tensor_reduce(sm[:], ex[:], mybir.AxisListType.X,
                                    AluOp.add)
            ls = sb.tile([G, 1], dt.float32, tag="ls")
            nc.scalar.activation(ls[:], sm[:], Act.Ln)
            outv = sb.tile([G, 2], dt.float32, tag="outv")
            nc.vector.tensor_scalar(outv[:], lg[:], ls[:, 0:1], None,
                                    AluOp.subtract)
            nc.sync.dma_start(out_d[:], outv[:])

    nc.compile()
    return nc


_last_result = [None]


def kernel(**inputs):
    import hashlib
    configure(int(np.asarray(inputs["x"]).shape[0]),
              int(np.asarray(inputs["edge_index"]).shape[1]))
    ek = np.ascontiguousarray(np.asarray(inputs["edge_index"]))
    bk = np.ascontiguousarray(np.asarray(inputs["batch"]))
    key = hashlib.sha1(ek.tobytes() + bk.tobytes()).hexdigest()
    in_maps, CH, choff, CH4, choff4, CHUNKS = _prep_host(**inputs)
    if key not in _cache:
        _cache[key] = _build(CH, choff, CH4, choff4, CHUNKS)
    nc = _cache[key]
    res = bass_utils.run_bass_kernel_spmd(nc, in_maps, core_ids=list(range(NC)))
    _last_result[0] = res
    return res.results[0]["out"].astype(np.float32)


def kernel_exec_ns():
    r = _last_result[0]
    return None if r is None else r.exec_time_ns


# revision 19
# speedup vs baseline: 1.0207x; 1.0207x over previous
"""GAT (2-layer, 4->1 heads) + global mean pool + classifier on 8 trn2 NeuronCores.

Sharding: nodes (and their incoming edges) partitioned contiguously across the
8 cores; small weights replicated; per-layer node-feature tables AllGathered
(in 4 chunks, overlapped with the producing phase); per-graph pooled sums
AllReduced.

Edge phase: per-edge feature rows pulled from the AllGathered DRAM table with
single-packet dma_gather calls (<=1024 indices each, rotated over 4 SWDGE
queues) so the SDMA engines pipeline the random HBM reads. The aggregation
one-hot is precomputed on the host and bulk-DMA'd; softmax weights ride along
the gathered rows and scale messages with a 2x-packed DVE multiply.

Self-contained: takes full inputs, returns full [64, 2] log-softmax output.
"""
import sys
for _p in ('/opt/trn_rl_repo', '/root/.axon_site/_ro/trn_rl_repo'):
    if _p not in sys.path:
        sys.path.insert(0, _p)

import heapq
import numpy as np
import concourse.bass as bass
import concourse.bacc as bacc
import concourse.tile as tile
import concourse.mybir as mybir
from concourse import bass_utils, library_config

dt = mybir.dt

# problem constants (hardcoded per contract); N/E re-derivable for sim tests
N = 50000
E = 1600000
G = 64
DIN = 128
HID = 64
H = 4
NEG_SLOPE = 0.2
BN_EPS = 1e-5
NC = 8
RF1 = 512                # L1 table row: fp8 slots (512B): 256 xws fp8 | 16B a_src(f32) | pad
RF2 = 128                # L2 table row: fp16 slots (256B): 64 xws | 2 a_src(f32) | pad
SPCH = 8                 # chunks per dma_gather call (8*128=1024 idx: 64-desc single packet max)
NAG = 5                  # AllGather chunks


def configure(n, e):
    global N, E, NSH, NT, NPAD, HALF, GRP, ROWS_G, BASE_G
    N, E = n, e
    NSH = N // NC
    NT = (NSH + 127) // 128
    NPAD = NT * 128
    HALF = NC // 2 * NPAD
    # AllGather tile groups, front-loaded so the final chunk (which gates the
    # next phase) is small
    base = -(-NT * 10 // (NAG * 10))
    big = min(NT - (NAG - 1), base + (base + 1) // 4)
    sizes = [big] * (NAG - 1) + [NT - big * (NAG - 1)]
    assert sizes[-1] >= 1
    GRP = [0]
    for s in sizes:
        GRP.append(GRP[-1] + s)
    ROWS_G = [(GRP[i + 1] - GRP[i]) * 128 for i in range(NAG)]
    BASE_G = [0]
    for rg in ROWS_G:
        BASE_G.append(BASE_G[-1] + NC * rg)


configure(N, E)

_cache = {}


def _balanced_assign(indeg):
    """Greedy LPT: nodes -> (core,tile) buckets, balancing edge counts.

    Returns newid[node] = core*NSH + tile*128 + slot."""
    nbuck = NC * NT
    lastcap = NSH - (NT - 1) * 128
    caps = np.full(nbuck, 128, np.int64)
    caps[NT - 1::NT] = lastcap
    order = np.argsort(-indeg, kind='stable')
    load = [0] * nbuck
    fill = [0] * nbuck
    heap = [(0, b) for b in range(nbuck)]
    heapq.heapify(heap)
    newid = np.empty(N, np.int64)
    caps_l = caps.tolist()
    indeg_l = indeg.tolist()
    for node in order.tolist():
        while True:
            l, b = heapq.heappop(heap)
            if fill[b] < caps_l[b]:
                break
        s = fill[b]
        fill[b] = s + 1
        nl = l + indeg_l[node]
        load[b] = nl
        if s + 1 < caps_l[b]:
            heapq.heappush(heap, (nl, b))
        c, t = divmod(b, NT)
        newid[node] = c * NSH + t * 128 + s
    return newid


def _prep_host(x, edge_index, batch,
               W1, att_src1, att_dst1, bias1, bn1_g, bn1_b, bn1_m, bn1_v,
               W2, att_src2, att_dst2, bias2, bn2_g, bn2_b, bn2_m, bn2_v,
               Wc1, bc1, Wc2, bc2):
    """Index-space layout + folded weights. Returns (in_maps, CH, choff, CHUNKS)."""
    f32 = np.float32
    src = np.concatenate([np.asarray(edge_index[0], np.int64),
                          np.arange(N, dtype=np.int64)])
    dst = np.concatenate([np.asarray(edge_index[1], np.int64),
                          np.arange(N, dtype=np.int64)])
    EE = src.shape[0]

    indeg = np.bincount(dst, minlength=N)
    newid = _balanced_assign(indeg)
    inv = np.empty(N, np.int64)
    inv[newid] = np.arange(N)      # inv[new] = old
    src = newid[src]
    dst = newid[dst]

    core = dst // NSH
    ldst = dst - core * NSH
    t = ldst >> 7                     # dst tile within shard
    dit = ldst & 127                  # dst index within tile

    # table row of src in the group-major AllGathered table
    score = src // NSH
    sr = src - score * NSH            # local row
    stile = sr >> 7
    g_of_tile = np.zeros(NT, np.int64)
    for gi in range(NAG):
        g_of_tile[GRP[gi]:GRP[gi + 1]] = gi
    sg = g_of_tile[stile]
    rows_g = np.asarray(ROWS_G, np.int64)
    base_g = np.asarray(BASE_G[:NAG], np.int64)
    grp0 = np.asarray([GRP[i] * 128 for i in range(NAG)], np.int64)
    trow = base_g[sg] + score * rows_g[sg] + (sr - grp0[sg])
    g = (trow >= HALF).astype(np.int64)           # table half (int16 range)
    lidx = trow - g * HALF
    assert lidx.max() < 32768

    key = ((core * NT + t) * 2 + g)   # bucket id, core-major
    nbuck = NC * NT * 2
    cnt = np.bincount(key, minlength=nbuck).reshape(NC, NT, 2)
    CH = np.maximum(1, (cnt.max(axis=0) + 127) // 128)   # [NT, 2] shared chunks
    CHUNKS = int(CH.sum())
    choff = np.zeros((NT, 2), np.int64)
    choff.reshape(-1)[1:] = np.cumsum(CH.reshape(-1))[:-1]
    CH4 = CH
    choff4 = choff

    # stable-sort edges by bucket; ranks within bucket
    order = np.argsort(key, kind='stable')
    skey = key[order]
    bstart = np.searchsorted(skey, np.arange(nbuck))
    rank = np.arange(EE, dtype=np.int64) - bstart[skey]
    bt = (skey // 2) % NT
    bg = skey % 2
    pos = choff[bt, bg] * 128 + rank
    scor = skey // (NT * 2)

    gidx_all = np.zeros((NC, CHUNKS * 128), np.int16)
    dflat_all = np.full((NC, CHUNKS * 128), 999.0, np.float16)
    didx_all = np.full((NC, CHUNKS * 128), 999, np.int32)
    for c in range(NC):
        m = scor == c
        eidx = order[m]
        gidx_all[c, pos[m]] = lidx[eidx].astype(np.int16)
        dflat_all[c, pos[m]] = dit[eidx].astype(np.float16)
        didx_all[c, pos[m]] = dit[eidx]

    # gather index stream: wrapped in 16 partitions, replicated x8
    gidx = np.tile(gidx_all.reshape(NC, CHUNKS * 8, 16).transpose(0, 2, 1), (1, 8, 1)).copy()
    # host-built one-hots (fp8: 0.0/1.0 are exact)
    #   oh[p, ch, j]  = (dit of edge (p,ch) == j)     aggregation lhsT
    #   ohT[d, ch, e] = (dit of edge (e,ch) == d)     alpha_dst lhsT
    import ml_dtypes
    f8 = ml_dtypes.float8_e4m3fn
    ar128 = np.arange(128, dtype=np.int32)
    oh = np.empty((NC, 128, CHUNKS, 128), f8)
    ohT = np.empty((NC, 128, CHUNKS, 128), f8)
    for c in range(NC):
        dit_pc = didx_all[c].reshape(CHUNKS, 128)            # [ch, p]
        oh[c] = (dit_pc.T[:, :, None] == ar128[None, None, :]).astype(f8)
        ohT[c] = (ar128[:, None, None] == dit_pc[None, :, :]).astype(f8)

    # batch / pooling (note: node n' holds old node inv[n'])
    batch = np.asarray(batch, np.int64)[inv]
    bcol = np.full((NC, 128, NT), 999.0, np.float16)
    for c in range(NC):
        bc_ = batch[c * NSH:(c + 1) * NSH].astype(np.float16)
        pad = np.full(NPAD - NSH, 999.0, np.float16)
        bcol[c] = np.concatenate([bc_, pad]).reshape(NT, 128).T
    cnt_g = np.bincount(batch, minlength=G).astype(f32)
    cntrecip = (1.0 / np.maximum(cnt_g, 1.0)).reshape(G, 1)

    # folded weights
    W1 = np.asarray(W1, f32); W2 = np.asarray(W2, f32)
    s1 = np.asarray(bn1_g, f32) / np.sqrt(np.asarray(bn1_v, f32) + BN_EPS)
    t1 = (np.asarray(bias1, f32) - np.asarray(bn1_m, f32)) * s1 + np.asarray(bn1_b, f32)
    s2 = np.asarray(bn2_g, f32) / np.sqrt(np.asarray(bn2_v, f32) + BN_EPS)
    t2 = (np.asarray(bias2, f32) - np.asarray(bn2_m, f32)) * s2 + np.asarray(bn2_b, f32)
    aS1 = np.asarray(att_src1, f32)   # [H, HID]
    aD1 = np.asarray(att_dst1, f32)
    Ablk = np.zeros((H * HID, 2 * H), f32)
    for h in range(H):
        Ablk[h * HID:(h + 1) * HID, h] = aS1[h]
        Ablk[h * HID:(h + 1) * HID, H + h] = aD1[h]
    W1e = np.concatenate([W1 * s1[None, :], W1 @ Ablk], axis=1)      # [128, 264]
    t1row = np.concatenate([t1, np.zeros(2 * H, f32)]).reshape(1, 264)
    aS2 = np.asarray(att_src2, f32).reshape(HID)
    aD2 = np.asarray(att_dst2, f32).reshape(HID)
    W2e = np.concatenate([W2 * s2[None, :], (W2 @ aS2)[:, None],
                          (W2 @ aD2)[:, None]], axis=1)              # [256, 66]
    t2row = np.concatenate([t2, np.zeros(2, f32)]).reshape(1, 66)

    iota64 = np.tile(np.arange(64, dtype=np.float16), (128, 1))
    ident = np.eye(128, dtype=f32)
    onesrow = np.ones((1, 128), f32)

    x = np.asarray(x, f32)[inv]
    in_maps = []
    for c in range(NC):
        xs = x[c * NSH:(c + 1) * NSH]
        xT = np.zeros((DIN, NPAD), np.float16)
        xT[:, :NSH] = xs.T.astype(np.float16)
        in_maps.append({
            "xT": xT, "gidx": gidx[c],
            "oh": oh[c], "ohT": ohT[c],
            "bcol": bcol[c].copy(),
            "W1e": W1e.astype(np.float16), "t1row": t1row,
            "W2e": W2e.reshape(2, 128, 66).transpose(1, 0, 2).copy(),
            "t2row": t2row,
            "iota64": iota64, "ident": ident, "onesrow": onesrow,
            "Wc1": np.asarray(Wc1, f32), "bc1row": np.asarray(bc1, f32).reshape(1, HID),
            "Wc2": np.asarray(Wc2, f32), "bc2row": np.asarray(bc2, f32).reshape(1, 2),
            "cntrecip": cntrecip,
        })
    return in_maps, CH, choff, CH4, choff4, CHUNKS


def _build(CH, choff, CH4, choff4, CHUNKS):
    AluOp = mybir.AluOpType
    Act = mybir.ActivationFunctionType
    nc = bacc.Bacc("TRN2", target_bir_lowering=False, debug=False, num_devices=NC,
                   num_swdge_queues=4)

    xT_d = nc.dram_tensor("xT", [DIN, NPAD], dt.float16, kind="ExternalInput")
    gidx_d = nc.dram_tensor("gidx", [128, CHUNKS * 8], dt.int16, kind="ExternalInput")
    oh_d = nc.dram_tensor("oh", [128, CHUNKS, 128], dt.float8e4, kind="ExternalInput")
    ohT_d = nc.dram_tensor("ohT", [128, CHUNKS, 128], dt.float8e4, kind="ExternalInput")
    bcol_d = nc.dram_tensor("bcol", [128, NT], dt.float16, kind="ExternalInput")
    W1e_d = nc.dram_tensor("W1e", [DIN, 264], dt.float16, kind="ExternalInput")
    t1row_d = nc.dram_tensor("t1row", [1, 264], dt.float32, kind="ExternalInput")
    W2e_d = nc.dram_tensor("W2e", [128, 2, 66], dt.float32, kind="ExternalInput")
    t2row_d = nc.dram_tensor("t2row", [1, 66], dt.float32, kind="ExternalInput")
    iota64_d = nc.dram_tensor("iota64", [128, 64], dt.float16, kind="ExternalInput")
    ident_d = nc.dram_tensor("ident", [128, 128], dt.float32, kind="ExternalInput")
    ones_d = nc.dram_tensor("onesrow", [1, 128], dt.float32, kind="ExternalInput")
    Wc1_d = nc.dram_tensor("Wc1", [HID, HID], dt.float32, kind="ExternalInput")
    bc1_d = nc.dram_tensor("bc1row", [1, HID], dt.float32, kind="ExternalInput")
    Wc2_d = nc.dram_tensor("Wc2", [HID, 2], dt.float32, kind="ExternalInput")
    bc2_d = nc.dram_tensor("bc2row", [1, 2], dt.float32, kind="ExternalInput")
    crec_d = nc.dram_tensor("cntrecip", [G, 1], dt.float32, kind="ExternalInput")
    out_d = nc.dram_tensor("out", [G, 2], dt.float32, kind="ExternalOutput")

    RG = [list(range(NC))]
    CTMAX = int((CH[:, 0] + CH[:, 1]).max())
    qrr = [0]

    with tile.TileContext(nc) as tc:
        with (
            tc.tile_pool(name="const", bufs=1) as cp,
            tc.tile_pool(name="sb", bufs=3) as sb,
            tc.tile_pool(name="gbuf", bufs=4) as gp,
            tc.tile_pool(name="ohbuf", bufs=4) as ohp,
            tc.tile_pool(name="gbuf2", bufs=2) as gp2,
            tc.tile_pool(name="small", bufs=4) as sp,
            tc.tile_pool(name="ps", bufs=2, space="PSUM") as ps,
            tc.tile_pool(name="pspool", bufs=1, space="PSUM") as psp,
            tc.tile_pool(name="dram", bufs=1, space="DRAM") as dram,
        ):
            nc.gpsimd.load_library(library_config.mlp)

            # ---- consts to SBUF
            def cload(dten, shape, dtype):
                tl = cp.tile(shape, dtype, tag=dten.name)
                nc.sync.dma_start(tl[:], dten[:])
                return tl
            W1e = cload(W1e_d, [DIN, 264], dt.float16)
            t1row = cload(t1row_d, [1, 264], dt.float32)
            W2e = cload(W2e_d, [128, 2, 66], dt.float32)
            t2row = cload(t2row_d, [1, 66], dt.float32)
            iota64 = cload(iota64_d, [128, 64], dt.float16)
            ident = cload(ident_d, [128, 128], dt.float32)
            ones = cload(ones_d, [1, 128], dt.float32)
            Wc1 = cload(Wc1_d, [HID, HID], dt.float32)
            bc1row = cload(bc1_d, [1, HID], dt.float32)
            Wc2 = cload(Wc2_d, [HID, 2], dt.float32)
            bc2row = cload(bc2_d, [1, 2], dt.float32)
            cntrecip = cload(crec_d, [G, 1], dt.float32)
            bcol = cload(bcol_d, [128, NT], dt.float16)

            ad1 = cp.tile([128, NT, H], dt.float16, tag="ad1")
            ad2 = cp.tile([128, NT, 1], dt.float16, tag="ad2")
            poh = cp.tile([128, NT, G], dt.float16, tag="poh")
            # pooling one-hot (built once)
            nc.vector.tensor_tensor(
                poh[:],
                iota64[:].unsqueeze(1).broadcast_to([128, NT, G]),
                bcol[:].unsqueeze(2).broadcast_to([128, NT, G]),
                AluOp.is_equal)

            # ---- DRAM tables (collective outputs in Shared space)
            t1stage = dram.tile([NPAD, RF1], dt.float8e4)
            t1full = nc.dram_tensor("t1full", [NC * NPAD, RF1], dt.float8e4,
                                    kind="Internal", addr_space="Shared").ap()
            t2stage = dram.tile([NPAD, RF2], dt.float16)
            t2full = nc.dram_tensor("t2full", [NC * NPAD, RF2], dt.float16,
                                    kind="Internal", addr_space="Shared").ap()

            def ag_chunk(stage, full, rfw, gi):
                a, b = GRP[gi] * 128, GRP[gi + 1] * 128
                base = BASE_G[gi]
                nc.gpsimd.collective_compute(
                    "AllGather", mybir.AluOpType.bypass, replica_groups=RG,
                    ins=[stage[a:b, :].opt()],
                    outs=[full[base:base + NC * (b - a), :].opt()])

            # ================= PHASE A: L1 head (xw1 table + alphas) ======
            with nc.named_scope("phaseA"), tc.tile_pool(name="head", bufs=2) as hp:
                for gi in range(NAG):
                    a, b = GRP[gi], GRP[gi + 1]
                    xTg = hp.tile([DIN, (b - a) * 128], dt.float16, tag="xTg")
                    nc.sync.dma_start(xTg[:], xT_d[:, a * 128:b * 128])
                    for t in range(a, b):
                        pa = ps.tile([128, 264], dt.float32, tag="pb")
                        nc.tensor.matmul(pa[:], xTg[:, (t - a) * 128:(t - a + 1) * 128],
                                         W1e[:],
                                         start=True, stop=False)
                        nc.tensor.matmul(pa[:], ones[0:1, :], t1row[:],
                                         start=False, stop=True)
                        tab = sb.tile([128, RF1], dt.float8e4, tag="tab1")
                        nc.scalar.activation(tab[:, 0:256], pa[:, 0:256], Act.Copy)
                        nc.vector.tensor_copy(tab[:, 256:272].bitcast(dt.float32),
                                              pa[:, 256:260])
                        nc.vector.tensor_copy(ad1[:, t, :], pa[:, 260:264])
                        nc.vector.memset(tab[:, 272:RF1].bitcast(dt.float16), 0.0)
                        nc.sync.dma_start(t1stage[t * 128:(t + 1) * 128, :], tab[:])
                    ag_chunk(t1stage, t1full, RF1, gi)

            # ================= PHASE B: L1 edges + L2 head ================
            def edge_phase(layer, tfull, rfw, nh, adt, adrow_tag):
                """One GAT edge phase. Yields per-tile (t, h) results."""
                halves = (tfull[0:HALF, :], tfull[HALF:2 * HALF, :])
                ncol = nh * HID   # message feature cols (256 / 64)
                fp8 = layer == 1
                for t in range(NT):
                    ct0 = int(choff[t, 0]); n0 = int(CH[t, 0])
                    n1 = int(CH[t, 1])
                    ctot = n0 + n1

                    # stage this tile's gather indices + host one-hots
                    gix = sp.tile([128, CTMAX * 8], dt.int16, tag="gix")
                    nc.sync.dma_start(gix[:, 0:ctot * 8],
                                      gidx_d[:, ct0 * 8:(ct0 + ctot) * 8])
                    oht = ohp.tile([128, CTMAX, 128], dt.float8e4, tag="oht")
                    nc.sync.dma_start(oht[:, 0:ctot, :],
                                      oh_d[:, ct0:ct0 + ctot, :])
                    ohTt = ohp.tile([128, CTMAX, 128], dt.float8e4, tag="ohTt")
                    nc.sync.dma_start(ohTt[:, 0:ctot, :],
                                      ohT_d[:, ct0:ct0 + ctot, :])

                    gb = gp.tile([128, CTMAX, rfw],
                                 dt.float8e4 if fp8 else dt.float16, tag="gb")
                    for gi, nch in ((0, n0), (1, n1)):
                        boff = 0 if gi == 0 else n0
                        for so in range(0, nch, SPCH):
                            ns = min(SPCH, nch - so)
                            nc.gpsimd.dma_gather(
                                gb[:, boff + so:boff + so + ns, :], halves[gi],
                                gix[:, (boff + so) * 8:(boff + so + ns) * 8],
                                num_idxs=ns * 128, num_idxs_reg=ns * 128,
                                elem_size=rfw, queue_num=qrr[0] % 4,
                                single_packet=True)
                            qrr[0] += 1
                    if fp8:
                        # cast fp8 features -> f16 message buffer on ACT
                        mb = gp2.tile([128, CTMAX, ncol + nh], dt.float16,
                                      tag="mbuf")
                        nc.scalar.activation(mb[:, 0:ctot, 0:ncol],
                                             gb[:, 0:ctot, 0:ncol], Act.Copy)
                        asrc = gb[:, 0:ctot, ncol:ncol + 4 * nh].bitcast(dt.float32)
                    else:
                        mb = gb
                        asrc = gb[:, 0:ctot, ncol:ncol + 2 * nh].bitcast(dt.float32)
                    ade = ps.tile([128, CTMAX, nh], dt.float32, tag="pexp")
                    for c in range(ctot):
                        nc.tensor.matmul(
                            ade[:, c, :],
                            ohTt[:, c, :], adt[:, t, :],
                            start=True, stop=True)
                    # e = a_src + a_dst ; lrelu ; exp -> fp16 into mb
                    ee = sp.tile([128, CTMAX, nh], dt.float32, tag=f"ee{layer}")
                    nc.vector.tensor_tensor(
                        ee[:, 0:ctot, :], asrc,
                        ade[:, 0:ctot, :], AluOp.add)
                    nc.vector.scalar_tensor_tensor(
                        ee[:, 0:ctot, :], ee[:, 0:ctot, :], NEG_SLOPE,
                        ee[:, 0:ctot, :], AluOp.mult, AluOp.max)
                    nc.scalar.activation(mb[:, 0:ctot, ncol:ncol + nh],
                                         ee[:, 0:ctot, :], Act.Exp)
                    # duplicate weights for a 2x-packed multiply
                    expd = sp.tile([128, CTMAX, nh, 2], dt.float16, tag=f"xd{layer}")
                    nc.vector.tensor_copy(
                        expd[:, 0:ctot, :, :],
                        mb[:, 0:ctot, ncol:ncol + nh].unsqueeze(3)
                          .broadcast_to([128, ctot, nh, 2]))
                    # scale messages by exp (in place, pair-packed APs, per head)
                    for h in range(nh):
                        nc.vector.tensor_tensor(
                            mb[:, 0:ctot, h * HID:(h + 1) * HID].rearrange(
                                "p c (j two) -> p c j two", two=2),
                            mb[:, 0:ctot, h * HID:(h + 1) * HID].rearrange(
                                "p c (j two) -> p c j two", two=2),
                            expd[:, 0:ctot, h, :].unsqueeze(2)
                              .broadcast_to([128, ctot, HID // 2, 2]),
                            AluOp.mult)
                    # aggregate
                    pb = ps.tile([128, ncol + nh], dt.float32, tag="pagg")
                    for c in range(ctot):
                        nc.tensor.matmul(pb[:], oht[:, c, :],
                                         mb[:, c, 0:ncol + nh],
                                         start=(c == 0), stop=(c == ctot - 1))
                    # epilogue: h = relu(num / (den + eps))
                    den = sp.tile([128, nh], dt.float32, tag=f"den{layer}")
                    nc.vector.tensor_scalar(den[:], pb[:, ncol:ncol + nh],
                                            1e-16, None, AluOp.add)
                    rec = sp.tile([128, nh], dt.float32, tag=f"rec{layer}")
                    nc.vector.reciprocal(rec[:], den[:])
                    hsb = sb.tile([128, ncol], dt.float32, tag=f"h{layer}")
                    nc.vector.tensor_tensor(
                        hsb[:].rearrange("p (h f) -> p h f", h=nh),
                        pb[:, 0:ncol].rearrange("p (h f) -> p h f", h=nh),
                        rec[:].unsqueeze(2).broadcast_to([128, nh, HID]),
                        AluOp.mult)
                    nc.scalar.activation(hsb[:], hsb[:], Act.Relu)
                    yield t, hsb

            # L1 edge phase; fused L2 head per tile, AG2 chunked
            ag2_next = 0
            for t, h1 in edge_phase(1, t1full, RF1, H, ad1, "a1"):
                # transpose h1 [128, 256] -> two [128,128] slices
                h1T = sb.tile([128, 2, 128], dt.float32, tag="h1T")
                for k in range(2):
                    pt = ps.tile([128, 128], dt.float32, tag="pb")
                    nc.tensor.transpose(pt[:], h1[:, k * 128:(k + 1) * 128],
                                        ident[:])
                    nc.scalar.copy(h1T[:, k, :], pt[:])
                pc = ps.tile([128, 66], dt.float32, tag="pb")
                for k in range(2):
                    nc.tensor.matmul(pc[:], h1T[:, k, :], W2e[:, k, :],
                                     start=(k == 0), stop=False)
                nc.tensor.matmul(pc[:], ones[0:1, :], t2row[:],
                                 start=False, stop=True)
                tab2 = sb.tile([128, RF2], dt.float16, tag="tab2")
                nc.scalar.activation(tab2[:, 0:HID], pc[:, 0:HID], Act.Copy)
                nc.vector.tensor_copy(tab2[:, HID:HID + 2].bitcast(dt.float32),
                                      pc[:, HID:HID + 1])
                nc.vector.tensor_copy(ad2[:, t, :], pc[:, HID + 1:HID + 2])
                nc.vector.memset(tab2[:, HID + 2:RF2], 0.0)
                nc.sync.dma_start(t2stage[t * 128:(t + 1) * 128, :], tab2[:])
                while (ag2_next < NAG - 1
                       and t >= min(GRP[ag2_next + 1] - 1 + 4, NT - 1)):
                    ag_chunk(t2stage, t2full, RF2, ag2_next)
                    ag2_next += 1
            while ag2_next < NAG:
                ag_chunk(t2stage, t2full, RF2, ag2_next)
                ag2_next += 1

            # ================= PHASE C: L2 edges + pooling ================
            pgsum = psp.tile([G, HID], dt.float32, tag="pgsum")
            for t, h2 in edge_phase(2, t2full, RF2, 1, ad2, "a2"):
                h2h = sb.tile([128, HID], dt.float16, tag="h2h")
                nc.scalar.activation(h2h[:], h2[:], Act.Copy)
                nc.tensor.matmul(pgsum[:], poh[:, t, :], h2h[:],
                                 start=(t == 0), stop=(t == NT - 1))

            # ================= PHASE D: AllReduce + classifier ============
            ar_in = dram.tile([G, HID], dt.float32)
            ar_out = nc.dram_tensor("ar_out", [G, HID], dt.float32,
                                    kind="Internal", addr_space="Shared").ap()
            psum_sb = sb.tile([G, HID], dt.float32, tag="psum_sb")
            nc.vector.tensor_copy(psum_sb[:], pgsum[:])
            nc.sync.dma_start(ar_in[:], psum_sb[:])
            nc.gpsimd.collective_compute(
                "AllReduce", mybir.AluOpType.add, replica_groups=RG,
                ins=[ar_in.opt()], outs=[ar_out.opt()])
            rep = sb.tile([G, HID], dt.float32, tag="rep")
            nc.sync.dma_start(rep[:], ar_out[:])
            nc.vector.tensor_scalar(rep[:], rep[:], cntrecip[:, 0:1], None,
                                    AluOp.mult)
            # hc = relu(rep @ Wc1 + bc1)
            ptr = ps.tile([G, G], dt.float32, tag="pb")
            nc.tensor.transpose(ptr[:], rep[:], ident[0:G, 0:G])
            repT = sb.tile([G, G], dt.float32, tag="repT")
            nc.scalar.copy(repT[:], ptr[:])
            ph = ps.tile([G, HID], dt.float32, tag="pb")
            nc.tensor.matmul(ph[:], repT[:], Wc1[:], start=True, stop=False)
            nc.tensor.matmul(ph[:], ones[0:1, 0:G], bc1row[:],
                             start=False, stop=True)
            hc = sb.tile([G, HID], dt.float32, tag="hc")
            nc.scalar.activation(hc[:], ph[:], Act.Relu)
            pt2 = ps.tile([G, G], dt.float32, tag="pb")
            nc.tensor.transpose(pt2[:], hc[:], ident[0:G, 0:G])
            hcT = sb.tile([G, G], dt.float32, tag="hcT")
            nc.scalar.copy(hcT[:], pt2[:])
            pl = ps.tile([G, 2], dt.float32, tag="pb")
            nc.tensor.matmul(pl[:], hcT[:], Wc2[:], start=True, stop=False)
            nc.tensor.matmul(pl[:], ones[0:1, 0:G], bc2row[:],
                             start=False, stop=True)
            # log softmax over the 2 logits
            lg = sb.tile([G, 2], dt.float32, tag="lg")
            nc.vector.tensor_copy(lg[:], pl[:])
            mx = sb.tile([G, 1], dt.float32, tag="mx")
            nc.vector.tensor_reduce(mx[:], lg[:], mybir.AxisListType.X,
                                    AluOp.max)
            nc.vector.tensor_scalar(lg[:], lg[:], mx[:, 0:1], None,
                                    AluOp.subtract)
            ex = sb.tile([G, 2], dt.float32, tag="ex")
            nc.scalar.activation(ex[:], lg[:], Act.Exp)
            sm = sb.tile([G, 1], dt.float32, tag="sm")
            nc.vector.tensor_reduce(sm[:], ex[:], mybir.AxisListType.X,
                                    AluOp.add)
            ls = sb.tile([G, 1], dt.float32, tag="ls")
            nc.scalar.activation(ls[:], sm[:], Act.Ln)
            outv = sb.tile([G, 2], dt.float32, tag="outv")
            nc.vector.tensor_scalar(outv[:], lg[:], ls[:, 0:1], None,
                                    AluOp.subtract)
            nc.sync.dma_start(out_d[:], outv[:])

    nc.compile()
    return nc


_last_result = [None]


def kernel(**inputs):
    import hashlib
    configure(int(np.asarray(inputs["x"]).shape[0]),
              int(np.asarray(inputs["edge_index"]).shape[1]))
    ek = np.ascontiguousarray(np.asarray(inputs["edge_index"]))
    bk = np.ascontiguousarray(np.asarray(inputs["batch"]))
    key = hashlib.sha1(ek.tobytes() + bk.tobytes()).hexdigest()
    in_maps, CH, choff, CH4, choff4, CHUNKS = _prep_host(**inputs)
    if key not in _cache:
        _cache[key] = _build(CH, choff, CH4, choff4, CHUNKS)
    nc = _cache[key]
    res = bass_utils.run_bass_kernel_spmd(nc, in_maps, core_ids=list(range(NC)))
    _last_result[0] = res
    return res.results[0]["out"].astype(np.float32)


def kernel_exec_ns():
    r = _last_result[0]
    return None if r is None else r.exec_time_ns
